# revision 33
# baseline (speedup 1.0000x reference)
"""MultiHeadAttention (B=2, S=2048, D=1024, H=16) on 8 Trainium2 NeuronCores.

Sharding: core c -> batch b = c // 4, head group g = c % 4 (4 of 16 heads =
256 of the 1024 projection columns). Within a batch's 4-core group:

  P1:  k projection, head-major: kt[oc] [128, S] fp16 (2 heads stacked
       64+64 per oc tile).  Slices 2-3 are interleaved into the first
       attention block so only half the startup is serial.
  P2:  v projection in natural layout [128, kbp, hg, 2, 65] fp16 with a
       ones column at col 64 (softmax denominators fall out of the AV
       matmul for free), also interleaved into the first attention block.
  P3:  per 512-query slice: q projection (interleaved into the previous
       slice's attention - engine queues are in-order, so extra PE work
       must be issued inside the Scalar-bound loops to fill PE gaps; all
       x DMA loads are hoisted early to avoid head-of-line blocking), then
       per head pair (oc), per key-block pair (kbp):
         scores: 4 matmuls (K=64 per head; head A on PE rows 0-63 via
         tile_position (0,0), head B on rows 64-127 via (64,0)) into a
         1024-wide 2-bank PSUM tile; one ScalarE exp per head (FD=1024,
         scale 1/8, bias -2 which cancels in softmax) -> fp16 pt; AV
         accumulates ov[65, 512] over all 16 key blocks.
       normalize: denominator rows are copied out of PSUM (rafp cannot
       read PSUM and only runs at base partition 0), inverted with the
       fast approx reciprocal in one [65,512] op, cast fp16, broadcast
       across partitions with a small K=65 fp16 matmul against a 0/1
       selector, and multiplied into fp16 ctxT kept in SBUF.
       NOTE fp8 anywhere in the v/pt path fails the 2e-2 gate: peaked
       softmax rows make ctx ~= v[argmax], so quantization error does not
       average out (measured 3.5e-2 with fp8e4 v, 6.5e-2 with fp8e5 pt).
  P4:  output projection as fp16 partials: every core projects all 512
       rows of the slice against its own 256 ctx dims (stationary = ctx
       tiles, moving = its 256-row slice of Wo.T), adds bo/4, and
       ReduceScatters the [512, 1024] fp16 partial; rank r of the group
       receives rows [512 qs + 128 r, +128).  pout/rsout are per-slice
       tensors: a shared tensor serializes slice N+1's writes behind the
       RS still reading slice N (measured as a 71 us whole-device stall).
"""

import numpy as np

import concourse.bacc as bacc
import concourse.mybir as mybir
from concourse.tile import TileContext
from concourse.bass_utils import run_bass_kernel_spmd

F32 = mybir.dt.float32
F16 = mybir.dt.float16
# NOTE: fp8 for v or probs fails the 2e-2 gate: peaked softmax rows make
# ctx ~= v[argmax] so quantization error does NOT average out (measured
# 3.5e-2 with fp8e4 v alone, 6.5e-2 adding fp8e5 probs).

B, S, D = 2, 2048, 1024
H, DH = 16, 64
NCORES = 8
GROUPS = [[0, 1, 2, 3], [4, 5, 6, 7]]
HPG = 4            # heads per core
DG = HPG * DH      # 256 projection cols per core
IC = D // 128      # 8 contraction chunks for the projections
KC = S // 128      # 16 key blocks
KBP = KC // 2      # 8 key block pairs
VW = 65            # head dim + ones col at 64
EXP_BIAS = -2.0    # folded into exp; cancels in softmax, keeps fp8 in range

_NC_CACHE = {}
DEBUG = False


def _build_nc():
    nc = bacc.Bacc("TRN2", target_bir_lowering=False, num_devices=NCORES)

    xq = nc.dram_tensor("xq", [IC, 128, S], F16, kind="ExternalInput")
    xk = nc.dram_tensor("xk", [IC, 128, S], F16, kind="ExternalInput")
    xv = nc.dram_tensor("xv", [IC, 128, S], F16, kind="ExternalInput")
    wq = nc.dram_tensor("wq", [IC, 128, DG], F16, kind="ExternalInput")
    wk = nc.dram_tensor("wk", [IC, 128, DG], F16, kind="ExternalInput")
    wv = nc.dram_tensor("wv", [IC, 128, DG], F16, kind="ExternalInput")
    wo = nc.dram_tensor("wo", [2, 128, D], F16, kind="ExternalInput")
    bq2 = nc.dram_tensor("bq2", [2, 128], F32, kind="ExternalInput")
    bk2 = nc.dram_tensor("bk2", [2, 128], F32, kind="ExternalInput")
    bvb = nc.dram_tensor("bvb", [128, DG], F32, kind="ExternalInput")
    bob4 = nc.dram_tensor("bob4", [128, D], F32, kind="ExternalInput")
    sel = nc.dram_tensor("sel", [65, 128], F16, kind="ExternalInput")
    out = nc.dram_tensor("out", [4, 128, D], F16, kind="ExternalOutput")

    pout = [nc.dram_tensor(f"pout{i}", [512, D], F16) for i in range(4)]
    rsout = [nc.dram_tensor(f"rsout{i}", [128, D], F16) for i in range(4)]
    if DEBUG:
        dbg_pt = nc.dram_tensor("dbg_pt", [2, 128, 1024], F16,
                                kind="ExternalOutput")
        dbg_rec = nc.dram_tensor("dbg_rec", [65, 512], F32,
                                 kind="ExternalOutput")
        dbg_ctx = nc.dram_tensor("dbg_ctx", [128, 512], F16,
                                 kind="ExternalOutput")

    Exp = mybir.ActivationFunctionType.Exp

    with nc.allow_low_precision(reason="fp16/fp8 attention internals"), \
            TileContext(nc) as tc:
        with (
            tc.tile_pool(name="persist", bufs=1) as pers,
            tc.tile_pool(name="xin", bufs=8) as xin,
            tc.tile_pool(name="pt", bufs=8) as ptp,
            tc.tile_pool(name="small", bufs=4) as small,
            tc.tile_pool(name="ctxp", bufs=4) as ctxpool,
            tc.tile_pool(name="outp", bufs=3) as outp,
            tc.tile_pool(name="sc", bufs=2, space="PSUM") as ps_sc,
            tc.tile_pool(name="ov", bufs=2, space="PSUM") as ps_ov,
            tc.tile_pool(name="misc", bufs=2, space="PSUM") as ps_misc,
        ):
            # ---- persistent SBUF ----
            wq_sb = pers.tile([128, IC * DG], F16, tag="wq")
            wk_sb = pers.tile([128, IC * DG], F16, tag="wk")
            wv_sb = pers.tile([128, IC * DG], F16, tag="wv")
            wo_sb = pers.tile([128, 2 * D], F16, tag="wo")
            qt_sb = [pers.tile([128, S], F16, tag=f"qt{i}", name=f"qt{i}")
                     for i in range(2)]
            kt_sb = [pers.tile([128, S], F16, tag=f"kt{i}", name=f"kt{i}")
                     for i in range(2)]
            v_sb = pers.tile([128, KBP * HPG * 2 * VW], F16, tag="v")
            bq_sb = pers.tile([128, 2], F32, tag="bq")
            bk_sb = pers.tile([128, 2], F32, tag="bk")
            bvb_sb = pers.tile([128, DG], F32, tag="bvb")
            bob_sb = pers.tile([128, D], F32, tag="bob")
            sel_sb = pers.tile([65, 128], F16, tag="sel")
            rec16_sb = pers.tile([65, 512], F16, tag="rec16")
            rec_sb = pers.tile([65, 512], F32, tag="rec")
            dsum_sb = pers.tile([65, 512], F32, tag="dsum")
            warm_sb = pers.tile([1, 16], F32, tag="warm")
            ebias_sb = pers.tile([128, 1], F32, tag="ebias")

            nc.sync.dma_start(
                out=wk_sb.rearrange("p (k n) -> p k n", k=IC),
                in_=wk.rearrange("k p n -> p k n"),
            )
            nc.sync.dma_start(out=bk_sb[:], in_=bk2.rearrange("c p -> p c"))
            for wsb, wdr, kch in ((wq_sb, wq, IC), (wv_sb, wv, IC),
                                  (wo_sb, wo, 2)):
                nc.sync.dma_start(
                    out=wsb.rearrange("p (k n) -> p k n", k=kch),
                    in_=wdr.rearrange("k p n -> p k n"),
                )
            nc.sync.dma_start(out=bq_sb[:], in_=bq2.rearrange("c p -> p c"))
            nc.sync.dma_start(out=bvb_sb[:], in_=bvb[:])
            nc.sync.dma_start(out=bob_sb[:], in_=bob4[:])
            nc.sync.dma_start(out=sel_sb[:], in_=sel[:])
            nc.vector.memset(rec_sb[:], 0.0)
            nc.vector.memset(rec16_sb[:], 0.0)
            nc.vector.memset(dsum_sb[:], 1.0)
            nc.vector.memset(v_sb[:], 0.0)
            v5 = v_sb.rearrange("p (kp h two e) -> p kp h two e",
                                h=HPG, two=2, e=VW)
            nc.vector.memset(v5[:, :, :, :, DH], 1.0)
            nc.vector.memset(ebias_sb[:], EXP_BIAS)
            # pull the ACT exp table load into the startup phase
            nc.vector.memset(warm_sb[:], 1.0)
            nc.scalar.activation(warm_sb[:], warm_sb[:], Exp, scale=0.1)

            wq3 = wq_sb.rearrange("p (k n) -> p k n", k=IC)
            wk3 = wk_sb.rearrange("p (k n) -> p k n", k=IC)
            wv3 = wv_sb.rearrange("p (k n) -> p k n", k=IC)
            wo3 = wo_sb.rearrange("p (k n) -> p k n", k=2)

            def load_x(xdr, s4, nm):
                xt = xin.tile([128, IC * 512], F16, tag="x", name=nm)
                nc.sync.dma_start(
                    out=xt.rearrange("p (k n) -> p k n", k=IC),
                    in_=xdr[:, :, s4 * 512:(s4 + 1) * 512].rearrange(
                        "k p n -> p k n"),
                )
                return xt.rearrange("p (k n) -> p k n", k=IC)

            def qk_proj_oc(xt3, wsb3, bsb, dst, s4, oc):
                acc = ps_misc.tile([128, 512], F32, tag="mm",
                                   name=f"acc{s4}{oc}")
                for ic in range(IC):
                    nc.tensor.matmul(
                        acc[:],
                        wsb3[:, ic, oc * 128:(oc + 1) * 128],
                        xt3[:, ic, :],
                        start=(ic == 0),
                        stop=(ic == IC - 1),
                    )
                nc.vector.tensor_scalar_add(
                    dst[oc][:, s4 * 512:(s4 + 1) * 512], acc[:],
                    bsb[:, oc:oc + 1],
                )

            def qk_proj(xt3, wsb3, bsb, dst, s4):
                for oc in range(2):
                    qk_proj_oc(xt3, wsb3, bsb, dst, s4, oc)

            def v_proj_kb(xt3, kb):
                j = kb % 4
                acc = ps_misc.tile([128, 512], F32, tag="mm", name=f"vacc{kb}")
                for ic in range(IC):
                    nc.tensor.matmul(
                        acc[:, 0:DG],
                        xt3[:, ic, j * 128:(j + 1) * 128],
                        wv3[:, ic, :],
                        start=(ic == 0),
                        stop=(ic == IC - 1),
                    )
                nc.vector.tensor_add(
                    out=v5[:, kb // 2, :, kb % 2, 0:DH],
                    in0=acc[:, 0:DG].rearrange("p (h e) -> p h e", e=DH),
                    in1=bvb_sb.rearrange("p (h e) -> p h e", e=DH),
                )

            def v_proj_steps(xvt):
                """One step per key block, kb = 4..15 (0..3 are pre-rolled).
                Consumed 2 per kbp iteration, so v for key pair kbp is always
                issued ahead of the AV matmul that reads it."""
                for s4 in range(1, 4):
                    for j in range(4):
                        yield lambda xt3=xvt[s4], kb=4 * s4 + j: \
                            v_proj_kb(xt3, kb)

            def q_proj_steps(qs, xt3):
                """16 matmul/sink steps projecting query slice qs."""
                for oc in range(2):
                    acc = ps_misc.tile([128, 512], F32, tag="mm",
                                       name=f"qacc{qs}{oc}")
                    for ic in range(IC):
                        yield lambda oc=oc, ic=ic, acc=acc: nc.tensor.matmul(
                            acc[:],
                            wq3[:, ic, oc * 128:(oc + 1) * 128],
                            xt3[:, ic, :],
                            start=(ic == 0),
                            stop=(ic == IC - 1),
                        )
                    yield lambda oc=oc, acc=acc: nc.vector.tensor_scalar_add(
                        qt_sb[oc][:, qs * 512:(qs + 1) * 512], acc[:],
                        bq_sb[:, oc:oc + 1],
                    )

            def out_proj_steps(qs, ctx):
                """Partial output projection of slice qs: 512 rows against
                this core's 256 ctx dims (stationary = ctx tiles)."""
                for qb in range(4):
                    for oh in range(2):
                        def mm(qb=qb, oh=oh, acc_box=[None]):
                            acc = ps_misc.tile([128, 512], F32, tag="mm",
                                               name=f"oacc{qs}{qb}{oh}")
                            acc_box[0] = acc
                            for cc in range(2):
                                nc.tensor.matmul(
                                    acc[:],
                                    ctx[cc][:, qb * 128:(qb + 1) * 128],
                                    wo3[:, cc, oh * 512:(oh + 1) * 512],
                                    start=(cc == 0),
                                    stop=(cc == 1),
                                )
                            ot = outp.tile([128, 512], F16, tag="ot")
                            nc.vector.tensor_add(
                                out=ot[:], in0=acc[:],
                                in1=bob_sb[:, oh * 512:(oh + 1) * 512],
                            )
                            nc.sync.dma_start(
                                out=pout[qs][qb * 128:(qb + 1) * 128,
                                             oh * 512:(oh + 1) * 512],
                                in_=ot[:],
                            )
                        yield mm

            def attention(qs, oc, extra):
                """Attention for query slice qs, head pair oc; `extra` is an
                iterator of callables drained 2 per kbp iteration to fill
                PE gaps in this Scalar-bound loop."""
                qlo = qs * 512
                ov = [ps_ov.tile([VW, 512], F32, tag="ov",
                                 name=f"ov{qs}{oc}{h}") for h in range(2)]
                for kbp in range(KBP):
                    for fn in (next(extra, None), next(extra, None)):
                        if fn is not None:
                            fn()
                    sc = [ps_sc.tile([128, 1024], F32, tag="sc",
                                     name=f"sc{h}") for h in range(2)]
                    for tw in range(2):
                        kb = 2 * kbp + tw
                        for h in range(2):  # row-tiled: A/B run concurrently
                            nc.tensor.matmul(
                                sc[h][:, tw * 512:(tw + 1) * 512],
                                kt_sb[oc][64 * h:64 * h + 64,
                                          kb * 128:(kb + 1) * 128],
                                qt_sb[oc][64 * h:64 * h + 64, qlo:qlo + 512],
                                start=True,
                                stop=True,
                            )
                    for h in range(2):
                        pt = ptp.tile([128, 1024], F16, tag="pt")
                        nc.scalar.activation(pt[:], sc[h][:], Exp,
                                             scale=0.125, bias=ebias_sb[:])
                        if DEBUG and qs == 0 and oc == 0 and kbp == 0:
                            nc.sync.dma_start(out=dbg_pt[h], in_=pt[:])
                        for tw in range(2):
                            nc.tensor.matmul(
                                ov[h][:],
                                v5[:, kbp, 2 * oc + h, tw, :],
                                pt[:, tw * 512:(tw + 1) * 512],
                                start=(kbp == 0 and tw == 0),
                                stop=(kbp == KBP - 1 and tw == 1),
                            )
                for fn in extra:  # drain any leftover interleaved steps
                    fn()
                # normalize: 1/denominator, broadcast via a K=65 matmul.
                # recA sits at row 0, recB at row 64 (engine APs need a
                # 32-aligned base partition).  rafp can't read PSUM and only
                # runs at base partition 0, so: copy denom rows into the
                # persistent dsum tile (rows 1..63 stay 1.0) and invert all
                # 65 rows in one op; sel zeros out rows 1..63 in the matmul.
                for h in range(2):
                    nc.vector.tensor_copy(out=dsum_sb[64 * h:64 * h + 1, :],
                                          in_=ov[h][DH:DH + 1, :])
                nc.vector.reciprocal_approx_fast(
                    out=rec_sb[:], in_=dsum_sb[:])
                nc.vector.tensor_copy(out=rec16_sb[:], in_=rec_sb[:])
                bc = ps_misc.tile([128, 512], F32, tag="mm", name="bc")
                nc.tensor.matmul(bc[:], sel_sb[:], rec16_sb[:],
                                 start=True, stop=True)
                bcs = small.tile([128, 512], F32, tag="bcs")
                nc.vector.tensor_copy(out=bcs[:], in_=bc[:])
                ctxp = ctxpool.tile([128, 512], F16, tag="cx",
                                    name=f"cx{qs}{oc}")
                for h in range(2):
                    nc.vector.tensor_mul(
                        out=ctxp[64 * h:64 * h + 64, :],
                        in0=ov[h][0:DH, :],
                        in1=bcs[64 * h:64 * h + 64, :],
                    )
                if DEBUG and qs == 0 and oc == 0:
                    nc.sync.dma_start(out=dbg_rec[:], in_=rec_sb[:])
                    nc.sync.dma_start(out=dbg_ctx[:], in_=ctxp[:])
                return ctxp

            # ---- schedule ----
            # preroll: kt for s4 0-1, v for kb 0-3, all of qt slice 0.
            # kt s4 2-3 interleave into attention(0,0): scores for kbp k
            # only need kt slice k//2, and the steps stay ahead of that.
            xkt = [load_x(xk, s4, f"xk{s4}") for s4 in range(2)]
            for s4 in range(2):
                qk_proj(xkt[s4], wk3, bk_sb, kt_sb, s4)
            xk23 = [load_x(xk, s4, f"xk{s4}") for s4 in (2, 3)]
            xvt = [load_x(xv, s4, f"xv{s4}") for s4 in range(4)]
            for kb in range(4):
                v_proj_kb(xvt[0], kb)
            for fn in q_proj_steps(0, load_x(xq, 0, "xq0")):
                fn()

            def k23_steps():
                for i, s4 in enumerate((2, 3)):
                    for oc in range(2):
                        yield lambda xt3=xk23[i], oc=oc, s4=s4: qk_proj_oc(
                            xt3, wk3, bk_sb, kt_sb, s4, oc)

            def reduce_scatter(qs):
                nc.gpsimd.collective_compute(
                    "ReduceScatter",
                    mybir.AluOpType.add,
                    replica_groups=GROUPS,
                    ins=[pout[qs][:, :]],
                    outs=[rsout[qs][:, :]],
                )
                nc.sync.dma_start(out=out[qs], in_=rsout[qs][:, :])

            from itertools import chain as _chain
            empty = iter(())
            vsteps = _chain(k23_steps(), v_proj_steps(xvt))
            ctx = {}
            for qs in range(4):
                # issue the next slice's xq DMA early: its first projection
                # matmul is interleaved into attention(qs,1) and would
                # head-of-line-block the in-order PE queue if the load were
                # issued there.
                xt3n = load_x(xq, qs + 1, f"xq{qs + 1}") if qs < 3 else None
                c0 = attention(qs, 0,
                               vsteps if qs == 0 else
                               out_proj_steps(qs - 1, ctx.pop(qs - 1)))
                if qs > 0:
                    reduce_scatter(qs - 1)
                c1 = attention(qs, 1,
                               q_proj_steps(qs + 1, xt3n) if qs < 3 else empty)
                ctx[qs] = (c0, c1)
            for fn in out_proj_steps(3, ctx.pop(3)):
                fn()
            reduce_scatter(3)

    nc.compile()
    return nc


def _get_nc():
    if "nc" not in _NC_CACHE:
        _NC_CACHE["nc"] = _build_nc()
    return _NC_CACHE["nc"]


def _prep_inputs(Q, K, V, Wq, Wk, Wv, Wo, bq, bk, bv, bo):
    f = np.float32
    h = np.float16
    Q, K, V = (np.asarray(a, f) for a in (Q, K, V))
    Wq, Wk, Wv, Wo = (np.asarray(a, f) for a in (Wq, Wk, Wv, Wo))
    bq, bk, bv, bo = (np.asarray(a, f) for a in (bq, bk, bv, bo))

    xqs = [np.ascontiguousarray(Q[b].T).astype(h).reshape(IC, 128, S)
           for b in range(B)]
    xks = [np.ascontiguousarray(K[b].T).astype(h).reshape(IC, 128, S)
           for b in range(B)]
    xvs = [np.ascontiguousarray(V[b].T).astype(h).reshape(IC, 128, S)
           for b in range(B)]
    WqT, WkT, WvT, WoT = Wq.T, Wk.T, Wv.T, Wo.T
    bob4 = np.ascontiguousarray(np.broadcast_to(bo / 4.0, (128, D)), dtype=f)
    sel = np.zeros((65, 128), h)
    sel[0, 0:64] = 1.0
    sel[64, 64:128] = 1.0

    in_maps = []
    for c in range(NCORES):
        b, g = c // 4, c % 4
        cols = slice(DG * g, DG * (g + 1))
        in_maps.append({
            "xq": xqs[b], "xk": xks[b], "xv": xvs[b],
            "wq": np.ascontiguousarray(WqT[:, cols], dtype=h).reshape(IC, 128, DG),
            "wk": np.ascontiguousarray(WkT[:, cols], dtype=h).reshape(IC, 128, DG),
            "wv": np.ascontiguousarray(WvT[:, cols], dtype=h).reshape(IC, 128, DG),
            "wo": np.ascontiguousarray(WoT[cols, :], dtype=h).reshape(2, 128, D),
            "bq2": np.ascontiguousarray(bq[cols]).reshape(2, 128),
            "bk2": np.ascontiguousarray(bk[cols]).reshape(2, 128),
            "bvb": np.ascontiguousarray(np.broadcast_to(bv[cols], (128, DG))),
            "bob4": bob4,
            "sel": sel,
        })
    return in_maps


def _assemble(results):
    out = np.empty((B, S, D), np.float32)
    for c in range(NCORES):
        b, g = c // 4, c % 4
        for qs in range(4):
            out[b, qs * 512 + g * 128:qs * 512 + (g + 1) * 128, :] = \
                results[c]["out"][qs].astype(np.float32)
    return out


def kernel(**inputs):
    nc = _get_nc()
    in_maps = _prep_inputs(**inputs)
    res = run_bass_kernel_spmd(nc, in_maps, core_ids=list(range(NCORES)))
    return _assemble(res.results)
